# revision 29
# baseline (speedup 1.0000x reference)
"""Multi-head attention (RoPE, non-causal) on 8 Trainium2 cores.

Sharding: DP=2 over batch x TP=4 over heads (4 heads per core).
Each core computes, for its (batch, head-group):
    kT   = RoPE(x @ wk.T).T   [per head: 128 x S, head-dim on partitions]
    v    = x @ wv.T           [S x 512, tokens on partitions]
  then per 512-token q-block:
    qT     = RoPE(x @ wq.T).T            [bf16]
    expT   = exp(scale * k.T q)          [transposed scores, bf16]
    denom  = ones^T (wide-tile add tree of expT on DVE)  [1 PE matmul]
    avT    = (v.T @ expT) * recip(denom)
    yT_partial = woT.T-slice @ avT        [2048 x 512]
Host sums the 4 head-group partials per batch and transposes.

All matmul operands are bf16 (same PE column rate as fp32r, half the
SBUF/DMA traffic); accumulation stays fp32 in PSUM. Scheduling: the
out-projection of block sq, the last two AV matmuls of each head, and
each head's softmax denominator are all deferred into later PE streams
(next head / next block's qproj) so exp and DVE tails never stall the
PE; the reciprocal row is broadcast on gpsimd, y tiles stream out over
two DMA queues from two alternating PSUM rings.
"""

import sys
import types
import numpy as np

HIDDEN = 2048
NUM_HEADS = 16
HEAD_DIM = 128
ROPE_BASE = 10000.0
BATCH = 2
SEQ = 2048

N_CORES = 8
DP = 2            # batch shards
TP = 4            # head-group shards
HPC = NUM_HEADS // TP          # heads per core = 4
DPC = HPC * HEAD_DIM           # head dims per core = 512
P = 128                        # partitions
TB = 512                       # token block (matmul free dim)

FP8_AV = False                 # fp8 AV numerator costs ~3% error: too much
AVMUL_GPSIMD = False           # scalar-engine staging blocks exp: keep on DVE


def _ensure_axon_hooks():
    """bass_utils' trace path imports antenv.axon_hooks, which the container's
    antenv stub lacks. Provide it, backed by the ctypes NTFF hook."""
    import antenv

    if hasattr(antenv, "axon_hooks"):
        return
    try:
        from trn_agent_boot.trn_boot import _ntff_profile_via_ctypes

        hook = _ntff_profile_via_ctypes("/opt/axon/libaxon_pjrt.so")
    except Exception:
        hook = None
    m = types.ModuleType("antenv.axon_hooks")
    m.get_axon_ntff_profile_hook = lambda: hook
    sys.modules["antenv.axon_hooks"] = m
    antenv.axon_hooks = m


def build(seq=SEQ):
    """Build the per-core Bass program (SPMD: same program, per-core data)."""
    import concourse.tile as tile
    from concourse import bacc, mybir

    F32 = mybir.dt.float32
    F32R = mybir.dt.float32r
    BF16 = mybir.dt.bfloat16
    FP8 = mybir.dt.float8e4
    EX_DT = FP8 if FP8_AV else BF16
    EXP_BIAS = -2.0 if FP8_AV else 0.0
    DR = mybir.MatmulPerfMode.DoubleRow
    Exp = mybir.ActivationFunctionType.Exp

    HT = HIDDEN // P            # 16 hidden k-tiles
    NT = seq // TB              # token blocks (4)
    ST = seq // P               # 128-token k-tiles (16)
    NW = ST // 2                # wide (1024-col) score tiles per (h, sq) = 8
    scale = 1.0 / float(np.sqrt(HEAD_DIM))

    nc = bacc.Bacc("TRN2", target_bir_lowering=False, debug=False, num_devices=N_CORES)

    xT_d = nc.dram_tensor("xT", [HIDDEN, seq], BF16, kind="ExternalInput")
    wqT_d = nc.dram_tensor("wqT", [HIDDEN, DPC], BF16, kind="ExternalInput")
    wkT_d = nc.dram_tensor("wkT", [HIDDEN, DPC], BF16, kind="ExternalInput")
    wvT_d = nc.dram_tensor("wvT", [HIDDEN, DPC], BF16, kind="ExternalInput")
    woT_d = nc.dram_tensor("woT", [DPC, HIDDEN], BF16, kind="ExternalInput")
    cosT_d = nc.dram_tensor("cosT", [HEAD_DIM, seq], F32, kind="ExternalInput")
    sinT_d = nc.dram_tensor("sinT", [HEAD_DIM, seq], F32, kind="ExternalInput")
    yT_d = nc.dram_tensor("yT", [HIDDEN, seq], F32, kind="ExternalOutput")

    with tile.TileContext(nc) as tc, nc.allow_low_precision(
        reason="bf16/fp8 matmul operands; fp32 PSUM accumulation; tol is 2e-2"
    ), (
        tc.tile_pool(name="ones", bufs=1)
    ) as opool, tc.tile_pool(name="const", bufs=1) as cpool, (
        tc.tile_pool(name="kres", bufs=1)
    ) as kpool, tc.tile_pool(name="vres", bufs=1) as vpool, (
        tc.tile_pool(name="wq", bufs=1)
    ) as wqpool, tc.tile_pool(name="wo", bufs=1) as wopool, (
        tc.tile_pool(name="xsh", bufs=2)
    ) as xpool, tc.tile_pool(name="ropetmp", bufs=4) as rope_pool:
        ones_c = opool.tile([P, 1], BF16, tag="ones_c")   # denominator lhsT
        ones_r = opool.tile([1, P], F32R, tag="ones_r")   # replicate lhsT
        ones_f = opool.tile([P, P], F32, tag="ones_f")    # memset staging
        nc.vector.memset(ones_f[:], 1.0)
        nc.vector.tensor_copy(ones_c[:], ones_f[:, 0:1])
        nc.vector.tensor_copy(ones_r[:], ones_f[0:1, :])
        bias_sb = opool.tile([P, 1], F32, tag="bias_sb")
        nc.vector.memset(bias_sb[:], EXP_BIAS)

        cos_sb = cpool.tile([P, seq], F32, tag="cos")
        sin_sb = cpool.tile([P, seq], F32, tag="sin")
        kT = [kpool.tile([P, seq], BF16, tag=f"kT{h}", name=f"kT{h}") for h in range(HPC)]
        # v resident as one [128, kt, dims] tile so DoubleRow can take
        # [:, 2w:2w+2, head] as a [128, 2, 128] lhsT view.
        v_sb = vpool.tile([P, ST, DPC], EX_DT, tag="v", name="v")
        wq_sb = [wqpool.tile([P, DPC], BF16, tag=f"wq{i}", name=f"wq{i}") for i in range(HT)]
        wo_sb = [wopool.tile([P, HIDDEN], BF16, tag=f"wo{i}", name=f"wo{i}") for i in range(HPC)]

        def load_x(tb):
            ts = slice(TB * tb, TB * (tb + 1))
            xt = [xpool.tile([P, TB], BF16, tag=f"x{i}", name=f"x{i}") for i in range(HT)]
            for i in range(HT):
                nc.gpsimd.dma_start(out=xt[i][:], in_=xT_d[P * i : P * (i + 1), ts])
            return xt

        def rope(ps, dst, ts):
            """dst = ps*cos + rot(ps)*sin_signed  (sign folded into sinT)."""
            h2 = P // 2
            tmp = rope_pool.tile([P, TB], F32, tag="rt", name="rt")
            nc.vector.tensor_mul(tmp[0:h2, :], ps[h2:P, :], sin_sb[0:h2, ts])
            nc.vector.tensor_mul(tmp[h2:P, :], ps[0:h2, :], sin_sb[h2:P, ts])
            nc.vector.tensor_mul(dst, ps[:], cos_sb[:, ts])
            nc.vector.tensor_add(dst, dst, tmp[:])

        # ---- Phase 1: k & v projections (shared x load) ------------------
        # tb order [1,2,3,0] so x(tb=0) stays resident for phase 2's sq=0.
        TB_ORDER = [1, 2, 3, 0]
        x_resident = None
        with (
            tc.tile_pool(name="wkv", bufs=1) as wkvpool,
            tc.tile_pool(name="ps1", bufs=4, space="PSUM") as ps1,
        ):
            wk_sb = [wkvpool.tile([P, DPC], BF16, tag=f"wk{i}", name=f"wk{i}") for i in range(HT)]
            wv_sb = [wkvpool.tile([P, DPC], BF16, tag=f"wv{i}", name=f"wv{i}") for i in range(HT)]

            for n, tb in enumerate(TB_ORDER):
                ts = slice(TB * tb, TB * (tb + 1))
                xt = load_x(tb)
                if tb == 0:
                    x_resident = xt
                if n == 0:
                    # sync-queue weight/const loads, most-urgent first
                    for i in range(HT):
                        eng = nc.sync if i % 2 == 0 else nc.scalar
                        eng.dma_start(out=wk_sb[i][:], in_=wkT_d[P * i : P * (i + 1), :])
                    nc.sync.dma_start(out=cos_sb[:, ts], in_=cosT_d[:, ts])
                    nc.sync.dma_start(out=sin_sb[:, ts], in_=sinT_d[:, ts])
                    for i in range(HT):
                        eng = nc.sync if i % 2 == 0 else nc.scalar
                        eng.dma_start(out=wv_sb[i][:], in_=wvT_d[P * i : P * (i + 1), :])
                    for t2 in TB_ORDER[1:]:
                        t2s = slice(TB * t2, TB * (t2 + 1))
                        nc.sync.dma_start(out=cos_sb[:, t2s], in_=cosT_d[:, t2s])
                        nc.sync.dma_start(out=sin_sb[:, t2s], in_=sinT_d[:, t2s])
                    for i in range(HT):
                        nc.sync.dma_start(out=wq_sb[i][:], in_=wqT_d[P * i : P * (i + 1), :])
                    for i in range(HPC):
                        nc.sync.dma_start(out=wo_sb[i][:], in_=woT_d[P * i : P * (i + 1), :])
                for d in range(HPC):          # k projection + RoPE
                    ps = ps1.tile([P, TB], F32, tag="ps1", name="ps1")
                    for i in range(HT):
                        nc.tensor.matmul(
                            ps[:],
                            wk_sb[i][:, P * d : P * (d + 1)],
                            xt[i][:],
                            start=(i == 0),
                            stop=(i == HT - 1),
                        )
                    rope(ps, kT[d][:, ts], ts)
                for t in range(TB // P):      # v projection (tokens on psum parts)
                    ps = ps1.tile([P, DPC], F32, tag="ps1", name="ps1")
                    for i in range(HT):
                        nc.tensor.matmul(
                            ps[:],
                            xt[i][:, P * t : P * (t + 1)],
                            wv_sb[i][:],
                            start=(i == 0),
                            stop=(i == HT - 1),
                        )
                    nc.scalar.copy(v_sb[:, tb * (TB // P) + t, :], ps[:])

        # ---- Phase 2: per q-block: qproj -> scores/exp/AV -> out-proj ----
        with (
            tc.tile_pool(name="qblk", bufs=2) as qpool,
            tc.tile_pool(name="expp", bufs=2) as epool,
            tc.tile_pool(name="esum", bufs=2) as spool,
            tc.tile_pool(name="avres", bufs=2) as avpool,
            tc.tile_pool(name="recp", bufs=2) as recpool,
            tc.tile_pool(name="ybuf", bufs=8) as ypool,
            tc.tile_pool(name="psQD", bufs=2, space="PSUM") as psQD,
            tc.tile_pool(name="pss", bufs=2, space="PSUM") as pss,
            tc.tile_pool(name="psav", bufs=2, space="PSUM") as psav,
        ):
            def denom_start(es):
                """ones-matmul (denominator row) + 1-row fast reciprocal.
                The ones output borrows row 0 of a qproj-ring PSUM tile; the
                ring partner (next qproj/outproj group) only needs the slot
                back after the reciprocal, 0.7us later."""
                psX = psQD.tile([P, TB], F32, tag="psQD", name="psQD")
                nc.tensor.matmul(psX[0:1, :], ones_c[:], es[:], start=True, stop=True)
                rr1 = recpool.tile([1, TB], F32, tag="rr1", name="rr1")
                nc.vector.reciprocal_approx_fast(out=rr1[:], in_=psX[0:1, :])
                return rr1

            def denom_finish(rr1, ps_av, av):
                """broadcast the reciprocal row on gpsimd, scale av on DVE.
                No PE instruction at all, so the PE queue never blocks."""
                rs = recpool.tile([P, TB], F32, tag="rs", name="rs")
                nc.gpsimd.partition_broadcast(rs[:], rr1[:])
                nc.vector.tensor_mul(av[:], ps_av[:], rs[:])

            def qproj(sq, xt, pending):
                ts = slice(TB * sq, TB * (sq + 1))
                qT = [qpool.tile([P, TB], BF16, tag=f"qT{d}", name=f"qT{d}") for d in range(HPC)]
                rr1_p = None
                for d in range(HPC):
                    ps = psQD.tile([P, TB], F32, tag="psQD", name="psQD")
                    for i in range(HT):
                        nc.tensor.matmul(
                            ps[:],
                            wq_sb[i][:, P * d : P * (d + 1)],
                            xt[i][:],
                            start=(i == 0),
                            stop=(i == HT - 1),
                        )
                    rope(ps, qT[d][:], ts)
                    if pending is not None:
                        # head-3 tail of the previous block: its last two AV
                        # matmuls and denominator hide under the projections
                        if d == 0:
                            pending[3](NW - 2)
                            pending[3](NW - 1)
                            rr1_p = denom_start(pending[1])
                        elif d == 1:
                            denom_finish(rr1_p, pending[0], pending[2])
                return qT

            def attn(sq, qT):
                av = [avpool.tile([P, TB], BF16, tag=f"av{h}", name=f"av{h}") for h in range(HPC)]
                prev = None
                for h in range(HPC):
                    ex = [epool.tile([P, 2, TB], EX_DT, tag=f"e{w}", name=f"e{w}") for w in range(NW)]
                    ps_av = psav.tile([P, TB], F32, tag="psav", name="psav")
                    tA = spool.tile([P, 2, TB], BF16, tag="tA", name="tA")
                    tB = spool.tile([P, 2, TB], BF16, tag="tB", name="tB")
                    es = spool.tile([P, TB], BF16, tag="es", name="es")

                    def s_mm(w):
                        ps_s = pss.tile([P, 2, TB], F32, tag="pss", name="pss")
                        for half in range(2):
                            kt = 2 * w + half
                            nc.tensor.matmul(
                                ps_s[:, half, :],
                                kT[h][:, P * kt : P * (kt + 1)],
                                qT[h][:],
                                start=True,
                                stop=True,
                            )
                        nc.scalar.activation(ex[w][:], ps_s[:], Exp, scale=scale, bias=bias_sb[:])

                    # NB: default args pin this head's tiles — av_mm is
                    # called from the NEXT head's stream (deferred tail)
                    if FP8_AV:
                        def av_mm(w, ps_av=ps_av, ex=ex, h=h):
                            nc.tensor.matmul(
                                ps_av[:],
                                v_sb[:, 2 * w : 2 * w + 2, P * h : P * (h + 1)],
                                ex[w][:],
                                start=(w == 0),
                                stop=(w == NW - 1),
                                perf_mode=DR,
                            )
                    else:
                        def av_mm(w, ps_av=ps_av, ex=ex, h=h):
                            for half in range(2):
                                kt = 2 * w + half
                                nc.tensor.matmul(
                                    ps_av[:],
                                    v_sb[:, kt, P * h : P * (h + 1)],
                                    ex[w][:, half, :],
                                    start=(kt == 0),
                                    stop=(kt == ST - 1),
                                )

                    # software-pipelined: av trails exp by >=2 wides; the
                    # last two AV matmuls of head h-1 and its denominator all
                    # land inside head h's stream so the exp tail never
                    # stalls the PE.
                    s_mm(0)
                    s_mm(1)
                    av_mm(0)
                    s_mm(2)
                    nc.vector.tensor_add(tA[:], ex[0][:], ex[1][:])
                    s_mm(3)
                    if prev is not None:
                        prev[3](NW - 2)
                        prev[3](NW - 1)
                        rr1_p = denom_start(prev[1])
                    nc.vector.tensor_add(tB[:], ex[2][:], ex[3][:])
                    nc.vector.tensor_add(tA[:], tA[:], tB[:])
                    av_mm(1)
                    s_mm(4)
                    if prev is not None:
                        denom_finish(rr1_p, prev[0], prev[2])
                    av_mm(2)
                    s_mm(5)
                    nc.vector.tensor_add(tB[:], ex[4][:], ex[5][:])
                    nc.vector.tensor_add(tA[:], tA[:], tB[:])
                    av_mm(3)
                    s_mm(6)
                    av_mm(4)
                    s_mm(7)
                    nc.vector.tensor_add(tB[:], ex[6][:], ex[7][:])
                    nc.vector.tensor_add(tA[:], tA[:], tB[:])
                    nc.vector.tensor_add(es[:], tA[:, 0, :], tA[:, 1, :])
                    av_mm(5)
                    prev = (ps_av, es, av[h][:], av_mm)
                return av, prev

            def outproj(sq, av):
                ts = slice(TB * sq, TB * (sq + 1))
                for do in range(HIDDEN // P):
                    # alternate PSUM rings (the qproj ring is idle here), copy
                    # engines, and DMA queues: four psum slots + two copy
                    # engines + two queues sustain one 256KB tile per 853ns
                    pool = psav if do % 2 == 0 else psQD
                    tag = "psav" if do % 2 == 0 else "psQD"
                    ps = pool.tile([P, TB], F32, tag=tag, name=tag)
                    for i in range(HPC):
                        nc.tensor.matmul(
                            ps[:],
                            wo_sb[i][:, P * do : P * (do + 1)],
                            av[i][:],
                            start=(i == 0),
                            stop=(i == HPC - 1),
                        )
                    yt = ypool.tile([P, TB], F32, tag="yt", name="yt")
                    nc.vector.tensor_copy(yt[:], ps[:])
                    eng = nc.sync if do % 2 == 0 else nc.gpsimd
                    eng.dma_start(out=yT_d[P * do : P * (do + 1), ts], in_=yt[:])

            pend = None      # deferred out-projection
            pending_d = None # deferred head-3 denominator
            xt_cur = x_resident
            for sq in range(NT):
                qT = qproj(sq, xt_cur, pending_d)
                if pend is not None:
                    outproj(*pend)
                # x prefetch after outproj: its gpsimd issues must not sit
                # ahead of the odd-do y DMAs in the gpsimd queue
                xt_next = load_x(sq + 1) if sq + 1 < NT else None
                av, pending_d = attn(sq, qT)
                pend = (sq, av)
                xt_cur = xt_next
            pending_d[3](NW - 2)
            pending_d[3](NW - 1)
            rr1_f = denom_start(pending_d[1])
            denom_finish(rr1_f, pending_d[0], pending_d[2])
            outproj(*pend)

    nc.compile()
    return nc


def make_in_maps(hidden_states, wq, wk, wv, wo, seq=SEQ):
    """Host-side sharding: per-core input dict."""
    import ml_dtypes

    bf16 = ml_dtypes.bfloat16
    hs = np.asarray(hidden_states, dtype=np.float32)
    inv_freq = 1.0 / (ROPE_BASE ** (np.arange(0, HEAD_DIM, 2, dtype=np.float32) / HEAD_DIM))
    t = np.arange(seq, dtype=np.float32)
    freqs = np.outer(t, inv_freq)                       # [S, 64]
    emb = np.concatenate([freqs, freqs], axis=-1)       # [S, 128]
    cosT = np.ascontiguousarray(np.cos(emb).T, dtype=np.float32)   # [128, S]
    sinT = np.sin(emb).T.astype(np.float32)             # [128, S]
    sinT_signed = sinT.copy()
    sinT_signed[: HEAD_DIM // 2, :] *= -1.0             # rot sign folded in
    sinT_signed = np.ascontiguousarray(sinT_signed)

    xT = [np.ascontiguousarray(hs[b].T.astype(bf16)) for b in range(BATCH)]
    in_maps = []
    for c in range(N_CORES):
        b = c // TP
        g = c % TP
        rows = slice(DPC * g, DPC * (g + 1))
        in_maps.append(
            {
                "xT": xT[b],
                "wqT": np.ascontiguousarray(wq[rows, :].T.astype(bf16)),
                "wkT": np.ascontiguousarray(wk[rows, :].T.astype(bf16)),
                "wvT": np.ascontiguousarray(wv[rows, :].T.astype(bf16)),
                "woT": np.ascontiguousarray(wo[:, rows].T.astype(bf16)),
                "cosT": cosT,
                "sinT": sinT_signed,
            }
        )
    return in_maps


def combine_outputs(results, seq=SEQ):
    """Host-side unshard: sum head-group partials per batch, transpose."""
    y = np.zeros((BATCH, seq, HIDDEN), dtype=np.float32)
    for c in range(N_CORES):
        b = c // TP
        y[b] += results[c]["yT"].T
    return y


_NC_CACHE = {}


def kernel(hidden_states, wq, wk, wv, wo):
    _ensure_axon_hooks()
    from concourse.bass_utils import run_bass_kernel_spmd

    if "nc" not in _NC_CACHE:
        _NC_CACHE["nc"] = build(SEQ)
    nc = _NC_CACHE["nc"]
    in_maps = make_in_maps(hidden_states, wq, wk, wv, wo, SEQ)
    res = run_bass_kernel_spmd(nc, in_maps, core_ids=list(range(N_CORES)))
    return combine_outputs(res.results, SEQ)


# revision 30
# speedup vs baseline: 1.0103x; 1.0103x over previous
"""Multi-head attention (RoPE, non-causal) on 8 Trainium2 cores.

Sharding: DP=2 over batch x TP=4 over heads (4 heads per core).
Each core computes, for its (batch, head-group):
    kT   = RoPE(x @ wk.T).T   [per head: 128 x S, head-dim on partitions]
    v    = x @ wv.T           [S x 512, tokens on partitions]
  then per 512-token q-block:
    qT     = RoPE(x @ wq.T).T            [bf16]
    expT   = exp(scale * k.T q)          [transposed scores, bf16]
    denom  = ones^T (wide-tile add tree of expT on DVE)  [1 PE matmul]
    avT    = (v.T @ expT) * recip(denom)
    yT_partial = woT.T-slice @ avT        [2048 x 512]
Host sums the 4 head-group partials per batch and transposes.

All matmul operands are bf16 (same PE column rate as fp32r, half the
SBUF/DMA traffic); accumulation stays fp32 in PSUM. Scheduling: the
out-projection of block sq, the last two AV matmuls of each head, and
each head's softmax denominator are all deferred into later PE streams
(next head / next block's qproj) so exp and DVE tails never stall the
PE; the reciprocal row is broadcast on gpsimd, y tiles stream out over
two DMA queues from two alternating PSUM rings.
"""

import sys
import types
import numpy as np

HIDDEN = 2048
NUM_HEADS = 16
HEAD_DIM = 128
ROPE_BASE = 10000.0
BATCH = 2
SEQ = 2048

N_CORES = 8
DP = 2            # batch shards
TP = 4            # head-group shards
HPC = NUM_HEADS // TP          # heads per core = 4
DPC = HPC * HEAD_DIM           # head dims per core = 512
P = 128                        # partitions
TB = 512                       # token block (matmul free dim)

FP8_AV = False                 # fp8 AV numerator costs ~3% error: too much
AVMUL_GPSIMD = False           # scalar-engine staging blocks exp: keep on DVE


def _ensure_axon_hooks():
    """bass_utils' trace path imports antenv.axon_hooks, which the container's
    antenv stub lacks. Provide it, backed by the ctypes NTFF hook."""
    import antenv

    if hasattr(antenv, "axon_hooks"):
        return
    try:
        from trn_agent_boot.trn_boot import _ntff_profile_via_ctypes

        hook = _ntff_profile_via_ctypes("/opt/axon/libaxon_pjrt.so")
    except Exception:
        hook = None
    m = types.ModuleType("antenv.axon_hooks")
    m.get_axon_ntff_profile_hook = lambda: hook
    sys.modules["antenv.axon_hooks"] = m
    antenv.axon_hooks = m


def build(seq=SEQ):
    """Build the per-core Bass program (SPMD: same program, per-core data)."""
    import concourse.tile as tile
    from concourse import bacc, mybir

    F32 = mybir.dt.float32
    F32R = mybir.dt.float32r
    BF16 = mybir.dt.bfloat16
    FP8 = mybir.dt.float8e4
    EX_DT = FP8 if FP8_AV else BF16
    EXP_BIAS = -2.0 if FP8_AV else 0.0
    DR = mybir.MatmulPerfMode.DoubleRow
    Exp = mybir.ActivationFunctionType.Exp

    HT = HIDDEN // P            # 16 hidden k-tiles
    NT = seq // TB              # token blocks (4)
    ST = seq // P               # 128-token k-tiles (16)
    NW = ST // 2                # wide (1024-col) score tiles per (h, sq) = 8
    scale = 1.0 / float(np.sqrt(HEAD_DIM))

    nc = bacc.Bacc("TRN2", target_bir_lowering=False, debug=False, num_devices=N_CORES)

    xT_d = nc.dram_tensor("xT", [HIDDEN, seq], BF16, kind="ExternalInput")
    wqT_d = nc.dram_tensor("wqT", [HIDDEN, DPC], BF16, kind="ExternalInput")
    wkT_d = nc.dram_tensor("wkT", [HIDDEN, DPC], BF16, kind="ExternalInput")
    wvT_d = nc.dram_tensor("wvT", [HIDDEN, DPC], BF16, kind="ExternalInput")
    woT_d = nc.dram_tensor("woT", [DPC, HIDDEN], BF16, kind="ExternalInput")
    cosT_d = nc.dram_tensor("cosT", [HEAD_DIM, seq], F32, kind="ExternalInput")
    sinT_d = nc.dram_tensor("sinT", [HEAD_DIM, seq], F32, kind="ExternalInput")
    yT_d = nc.dram_tensor("yT", [HIDDEN, seq], F32, kind="ExternalOutput")

    with tile.TileContext(nc) as tc, nc.allow_low_precision(
        reason="bf16/fp8 matmul operands; fp32 PSUM accumulation; tol is 2e-2"
    ), (
        tc.tile_pool(name="ones", bufs=1)
    ) as opool, tc.tile_pool(name="const", bufs=1) as cpool, (
        tc.tile_pool(name="kres", bufs=1)
    ) as kpool, tc.tile_pool(name="vres", bufs=1) as vpool, (
        tc.tile_pool(name="wq", bufs=1)
    ) as wqpool, tc.tile_pool(name="wo", bufs=1) as wopool, (
        tc.tile_pool(name="xsh", bufs=2)
    ) as xpool, tc.tile_pool(name="ropetmp", bufs=4) as rope_pool:
        ones_c = opool.tile([P, 1], BF16, tag="ones_c")   # denominator lhsT
        ones_r = opool.tile([1, P], F32R, tag="ones_r")   # replicate lhsT
        ones_f = opool.tile([P, P], F32, tag="ones_f")    # memset staging
        nc.vector.memset(ones_f[:], 1.0)
        nc.vector.tensor_copy(ones_c[:], ones_f[:, 0:1])
        nc.vector.tensor_copy(ones_r[:], ones_f[0:1, :])
        bias_sb = opool.tile([P, 1], F32, tag="bias_sb")
        nc.vector.memset(bias_sb[:], EXP_BIAS)

        cos_sb = cpool.tile([P, seq], F32, tag="cos")
        sin_sb = cpool.tile([P, seq], F32, tag="sin")
        kT = [kpool.tile([P, seq], BF16, tag=f"kT{h}", name=f"kT{h}") for h in range(HPC)]
        # v resident as one [128, kt, dims] tile so DoubleRow can take
        # [:, 2w:2w+2, head] as a [128, 2, 128] lhsT view.
        v_sb = vpool.tile([P, ST, DPC], EX_DT, tag="v", name="v")
        wq_sb = [wqpool.tile([P, DPC], BF16, tag=f"wq{i}", name=f"wq{i}") for i in range(HT)]
        wo_sb = [wopool.tile([P, HIDDEN], BF16, tag=f"wo{i}", name=f"wo{i}") for i in range(HPC)]

        def load_x(tb):
            ts = slice(TB * tb, TB * (tb + 1))
            xt = [xpool.tile([P, TB], BF16, tag=f"x{i}", name=f"x{i}") for i in range(HT)]
            for i in range(HT):
                nc.gpsimd.dma_start(out=xt[i][:], in_=xT_d[P * i : P * (i + 1), ts])
            return xt

        def rope(ps, dst, ts):
            """dst = ps*cos + rot(ps)*sin_signed  (sign folded into sinT)."""
            h2 = P // 2
            tmp = rope_pool.tile([P, TB], F32, tag="rt", name="rt")
            nc.vector.tensor_mul(tmp[0:h2, :], ps[h2:P, :], sin_sb[0:h2, ts])
            nc.vector.tensor_mul(tmp[h2:P, :], ps[0:h2, :], sin_sb[h2:P, ts])
            nc.vector.tensor_mul(dst, ps[:], cos_sb[:, ts])
            nc.vector.tensor_add(dst, dst, tmp[:])

        # ---- Phase 1: k & v projections (shared x load) ------------------
        # tb order [1,2,3,0] so x(tb=0) stays resident for phase 2's sq=0.
        TB_ORDER = [1, 2, 3, 0]
        x_resident = None
        with (
            tc.tile_pool(name="wkv", bufs=1) as wkvpool,
            tc.tile_pool(name="ps1", bufs=4, space="PSUM") as ps1,
        ):
            wk_sb = [wkvpool.tile([P, DPC], BF16, tag=f"wk{i}", name=f"wk{i}") for i in range(HT)]
            wv_sb = [wkvpool.tile([P, DPC], BF16, tag=f"wv{i}", name=f"wv{i}") for i in range(HT)]

            for n, tb in enumerate(TB_ORDER):
                ts = slice(TB * tb, TB * (tb + 1))
                if n == 0:
                    # first x block split over two queues to halve the
                    # head-of-kernel arrival time
                    xt = [xpool.tile([P, TB], BF16, tag=f"x{i}", name=f"x{i}") for i in range(HT)]
                    for i in range(HT):
                        eng = nc.gpsimd if i % 2 == 0 else nc.scalar
                        eng.dma_start(out=xt[i][:], in_=xT_d[P * i : P * (i + 1), ts])
                else:
                    xt = load_x(tb)
                if tb == 0:
                    x_resident = xt
                if n == 0:
                    # sync-queue weight/const loads, most-urgent first
                    for i in range(HT):
                        nc.sync.dma_start(out=wk_sb[i][:], in_=wkT_d[P * i : P * (i + 1), :])
                    nc.sync.dma_start(out=cos_sb[:, ts], in_=cosT_d[:, ts])
                    nc.sync.dma_start(out=sin_sb[:, ts], in_=sinT_d[:, ts])
                    for i in range(HT):
                        nc.sync.dma_start(out=wv_sb[i][:], in_=wvT_d[P * i : P * (i + 1), :])
                    for t2 in TB_ORDER[1:]:
                        t2s = slice(TB * t2, TB * (t2 + 1))
                        nc.sync.dma_start(out=cos_sb[:, t2s], in_=cosT_d[:, t2s])
                        nc.sync.dma_start(out=sin_sb[:, t2s], in_=sinT_d[:, t2s])
                    for i in range(HT):
                        nc.sync.dma_start(out=wq_sb[i][:], in_=wqT_d[P * i : P * (i + 1), :])
                    for i in range(HPC):
                        nc.sync.dma_start(out=wo_sb[i][:], in_=woT_d[P * i : P * (i + 1), :])
                for d in range(HPC):          # k projection + RoPE
                    ps = ps1.tile([P, TB], F32, tag="ps1", name="ps1")
                    for i in range(HT):
                        nc.tensor.matmul(
                            ps[:],
                            wk_sb[i][:, P * d : P * (d + 1)],
                            xt[i][:],
                            start=(i == 0),
                            stop=(i == HT - 1),
                        )
                    rope(ps, kT[d][:, ts], ts)
                for t in range(TB // P):      # v projection (tokens on psum parts)
                    ps = ps1.tile([P, DPC], F32, tag="ps1", name="ps1")
                    for i in range(HT):
                        nc.tensor.matmul(
                            ps[:],
                            xt[i][:, P * t : P * (t + 1)],
                            wv_sb[i][:],
                            start=(i == 0),
                            stop=(i == HT - 1),
                        )
                    nc.scalar.copy(v_sb[:, tb * (TB // P) + t, :], ps[:])

        # ---- Phase 2: per q-block: qproj -> scores/exp/AV -> out-proj ----
        with (
            tc.tile_pool(name="qblk", bufs=2) as qpool,
            tc.tile_pool(name="expp", bufs=2) as epool,
            tc.tile_pool(name="esum", bufs=2) as spool,
            tc.tile_pool(name="avres", bufs=2) as avpool,
            tc.tile_pool(name="recp", bufs=2) as recpool,
            tc.tile_pool(name="ybuf", bufs=8) as ypool,
            tc.tile_pool(name="psQD", bufs=2, space="PSUM") as psQD,
            tc.tile_pool(name="pss", bufs=2, space="PSUM") as pss,
            tc.tile_pool(name="psav", bufs=2, space="PSUM") as psav,
        ):
            def denom_start(es):
                """ones-matmul (denominator row) + 1-row fast reciprocal.
                The ones output borrows row 0 of a qproj-ring PSUM tile; the
                ring partner (next qproj/outproj group) only needs the slot
                back after the reciprocal, 0.7us later."""
                psX = psQD.tile([P, TB], F32, tag="psQD", name="psQD")
                nc.tensor.matmul(psX[0:1, :], ones_c[:], es[:], start=True, stop=True)
                rr1 = recpool.tile([1, TB], F32, tag="rr1", name="rr1")
                nc.vector.reciprocal_approx_fast(out=rr1[:], in_=psX[0:1, :])
                return rr1

            def denom_finish(rr1, ps_av, av):
                """broadcast the reciprocal row on gpsimd, scale av on DVE.
                No PE instruction at all, so the PE queue never blocks."""
                rs = recpool.tile([P, TB], F32, tag="rs", name="rs")
                nc.gpsimd.partition_broadcast(rs[:], rr1[:])
                nc.vector.tensor_mul(av[:], ps_av[:], rs[:])

            def qproj(sq, xt, pending):
                ts = slice(TB * sq, TB * (sq + 1))
                qT = [qpool.tile([P, TB], BF16, tag=f"qT{d}", name=f"qT{d}") for d in range(HPC)]
                rr1_p = None
                for d in range(HPC):
                    ps = psQD.tile([P, TB], F32, tag="psQD", name="psQD")
                    for i in range(HT):
                        nc.tensor.matmul(
                            ps[:],
                            wq_sb[i][:, P * d : P * (d + 1)],
                            xt[i][:],
                            start=(i == 0),
                            stop=(i == HT - 1),
                        )
                    rope(ps, qT[d][:], ts)
                    if pending is not None:
                        # head-3 tail of the previous block: its last two AV
                        # matmuls and denominator hide under the projections
                        if d == 0:
                            pending[3](NW - 2)
                            pending[3](NW - 1)
                            rr1_p = denom_start(pending[1])
                        elif d == 1:
                            denom_finish(rr1_p, pending[0], pending[2])
                return qT

            def attn(sq, qT):
                av = [avpool.tile([P, TB], BF16, tag=f"av{h}", name=f"av{h}") for h in range(HPC)]
                prev = None
                for h in range(HPC):
                    ex = [epool.tile([P, 2, TB], EX_DT, tag=f"e{w}", name=f"e{w}") for w in range(NW)]
                    ps_av = psav.tile([P, TB], F32, tag="psav", name="psav")
                    tA = spool.tile([P, 2, TB], BF16, tag="tA", name="tA")
                    tB = spool.tile([P, 2, TB], BF16, tag="tB", name="tB")
                    es = spool.tile([P, TB], BF16, tag="es", name="es")

                    def s_mm(w):
                        ps_s = pss.tile([P, 2, TB], F32, tag="pss", name="pss")
                        for half in range(2):
                            kt = 2 * w + half
                            nc.tensor.matmul(
                                ps_s[:, half, :],
                                kT[h][:, P * kt : P * (kt + 1)],
                                qT[h][:],
                                start=True,
                                stop=True,
                            )
                        nc.scalar.activation(ex[w][:], ps_s[:], Exp, scale=scale, bias=bias_sb[:])

                    # NB: default args pin this head's tiles — av_mm is
                    # called from the NEXT head's stream (deferred tail)
                    if FP8_AV:
                        def av_mm(w, ps_av=ps_av, ex=ex, h=h):
                            nc.tensor.matmul(
                                ps_av[:],
                                v_sb[:, 2 * w : 2 * w + 2, P * h : P * (h + 1)],
                                ex[w][:],
                                start=(w == 0),
                                stop=(w == NW - 1),
                                perf_mode=DR,
                            )
                    else:
                        def av_mm(w, ps_av=ps_av, ex=ex, h=h):
                            for half in range(2):
                                kt = 2 * w + half
                                nc.tensor.matmul(
                                    ps_av[:],
                                    v_sb[:, kt, P * h : P * (h + 1)],
                                    ex[w][:, half, :],
                                    start=(kt == 0),
                                    stop=(kt == ST - 1),
                                )

                    # software-pipelined: av trails exp by >=2 wides; the
                    # last two AV matmuls of head h-1 and its denominator all
                    # land inside head h's stream so the exp tail never
                    # stalls the PE.
                    s_mm(0)
                    s_mm(1)
                    av_mm(0)
                    s_mm(2)
                    nc.vector.tensor_add(tA[:], ex[0][:], ex[1][:])
                    s_mm(3)
                    if prev is not None:
                        prev[3](NW - 2)
                        prev[3](NW - 1)
                        rr1_p = denom_start(prev[1])
                    nc.vector.tensor_add(tB[:], ex[2][:], ex[3][:])
                    nc.vector.tensor_add(tA[:], tA[:], tB[:])
                    av_mm(1)
                    s_mm(4)
                    if prev is not None:
                        denom_finish(rr1_p, prev[0], prev[2])
                    av_mm(2)
                    s_mm(5)
                    nc.vector.tensor_add(tB[:], ex[4][:], ex[5][:])
                    nc.vector.tensor_add(tA[:], tA[:], tB[:])
                    av_mm(3)
                    s_mm(6)
                    av_mm(4)
                    s_mm(7)
                    nc.vector.tensor_add(tB[:], ex[6][:], ex[7][:])
                    nc.vector.tensor_add(tA[:], tA[:], tB[:])
                    nc.vector.tensor_add(es[:], tA[:, 0, :], tA[:, 1, :])
                    av_mm(5)
                    prev = (ps_av, es, av[h][:], av_mm)
                return av, prev

            def outproj(sq, av):
                ts = slice(TB * sq, TB * (sq + 1))
                for do in range(HIDDEN // P):
                    # alternate PSUM rings (the qproj ring is idle here), copy
                    # engines, and DMA queues: four psum slots + two copy
                    # engines + two queues sustain one 256KB tile per 853ns
                    pool = psav if do % 2 == 0 else psQD
                    tag = "psav" if do % 2 == 0 else "psQD"
                    ps = pool.tile([P, TB], F32, tag=tag, name=tag)
                    for i in range(HPC):
                        nc.tensor.matmul(
                            ps[:],
                            wo_sb[i][:, P * do : P * (do + 1)],
                            av[i][:],
                            start=(i == 0),
                            stop=(i == HPC - 1),
                        )
                    yt = ypool.tile([P, TB], F32, tag="yt", name="yt")
                    # split copies across DVE and the (idle-here) scalar
                    # engine so the two PSUM rings drain independently
                    if do % 2 == 0:
                        nc.vector.tensor_copy(yt[:], ps[:])
                        nc.sync.dma_start(out=yT_d[P * do : P * (do + 1), ts], in_=yt[:])
                    else:
                        nc.scalar.copy(yt[:], ps[:])
                        nc.gpsimd.dma_start(out=yT_d[P * do : P * (do + 1), ts], in_=yt[:])

            pend = None      # deferred out-projection
            pending_d = None # deferred head-3 denominator
            xt_cur = x_resident
            for sq in range(NT):
                qT = qproj(sq, xt_cur, pending_d)
                if pend is not None:
                    outproj(*pend)
                # x prefetch after outproj: its gpsimd issues must not sit
                # ahead of the odd-do y DMAs in the gpsimd queue
                xt_next = load_x(sq + 1) if sq + 1 < NT else None
                av, pending_d = attn(sq, qT)
                pend = (sq, av)
                xt_cur = xt_next
            pending_d[3](NW - 2)
            pending_d[3](NW - 1)
            rr1_f = denom_start(pending_d[1])
            denom_finish(rr1_f, pending_d[0], pending_d[2])
            outproj(*pend)

    nc.compile()
    return nc


def make_in_maps(hidden_states, wq, wk, wv, wo, seq=SEQ):
    """Host-side sharding: per-core input dict."""
    import ml_dtypes

    bf16 = ml_dtypes.bfloat16
    hs = np.asarray(hidden_states, dtype=np.float32)
    inv_freq = 1.0 / (ROPE_BASE ** (np.arange(0, HEAD_DIM, 2, dtype=np.float32) / HEAD_DIM))
    t = np.arange(seq, dtype=np.float32)
    freqs = np.outer(t, inv_freq)                       # [S, 64]
    emb = np.concatenate([freqs, freqs], axis=-1)       # [S, 128]
    cosT = np.ascontiguousarray(np.cos(emb).T, dtype=np.float32)   # [128, S]
    sinT = np.sin(emb).T.astype(np.float32)             # [128, S]
    sinT_signed = sinT.copy()
    sinT_signed[: HEAD_DIM // 2, :] *= -1.0             # rot sign folded in
    sinT_signed = np.ascontiguousarray(sinT_signed)

    xT = [np.ascontiguousarray(hs[b].T.astype(bf16)) for b in range(BATCH)]
    in_maps = []
    for c in range(N_CORES):
        b = c // TP
        g = c % TP
        rows = slice(DPC * g, DPC * (g + 1))
        in_maps.append(
            {
                "xT": xT[b],
                "wqT": np.ascontiguousarray(wq[rows, :].T.astype(bf16)),
                "wkT": np.ascontiguousarray(wk[rows, :].T.astype(bf16)),
                "wvT": np.ascontiguousarray(wv[rows, :].T.astype(bf16)),
                "woT": np.ascontiguousarray(wo[:, rows].T.astype(bf16)),
                "cosT": cosT,
                "sinT": sinT_signed,
            }
        )
    return in_maps


def combine_outputs(results, seq=SEQ):
    """Host-side unshard: sum head-group partials per batch, transpose."""
    y = np.zeros((BATCH, seq, HIDDEN), dtype=np.float32)
    for c in range(N_CORES):
        b = c // TP
        y[b] += results[c]["yT"].T
    return y


_NC_CACHE = {}


def kernel(hidden_states, wq, wk, wv, wo):
    _ensure_axon_hooks()
    from concourse.bass_utils import run_bass_kernel_spmd

    if "nc" not in _NC_CACHE:
        _NC_CACHE["nc"] = build(SEQ)
    nc = _NC_CACHE["nc"]
    in_maps = make_in_maps(hidden_states, wq, wk, wv, wo, SEQ)
    res = run_bass_kernel_spmd(nc, in_maps, core_ids=list(range(N_CORES)))
    return combine_outputs(res.results, SEQ)
